# revision 40
# baseline (speedup 1.0000x reference)
"""Single-head attention (B=4, S=2048, D=E=1024) on 8 trn2 NeuronCores.

Two-launch tensor-parallel scheme (the sharding hint's Megatron-style split,
with the collective realized as the host gather/unshard step the kernel()
contract provides between launches):

  Launch 1 (proj): data-parallel over (batch, k-half). Each core computes
    the K/V projections for HALF of its batch's keys -- kp^T[:, k-half] and
    vp[k-half, :] -- so nothing is computed twice. 256 matmuls/core.
  Host: concatenates the two halves of each batch's kp/vp (the all-gather).
  Launch 2 (attn): data-parallel over (batch, q-half). Each core loads the
    full kp/vp for its batch plus its 1024-row q shard and runs
    qp -> logits -> softmax -> ctx -> out. 770 matmuls/core.

Both launches use the techniques that made the single-launch version run as
a gapless PE stream at 213 ns per 512-row bf16 matmul:
  - all-bf16 operands (same steady-state PE rate as f32r, half the bytes)
  - ONE psum pool with 8 fixed bank tags per module (per-phase pools would
    insert full-pool barrier waits on the PE sequencer; shared tags make
    the hazards per-bank WARs that resolve early)
  - drains alternate Act/DVE so the last drain gating a successor phase
    has ~0.7us latency, covered by the successor's first matmuls
  - tiny warm-up matmuls through the DMA-latency startup window so the
    cost model's PE clock ramp (0.65->1.2->2.4 GHz over 3us) completes
    before the real stream begins
  - one sync-queue DMA stream in need-order (FIFO self-throttles
    background loads behind the critical ones)
  - softmax sum via DVE-presummed exp tiles (1 matmul per q-block)
  - out projection fused per q-block; final tile folds ob via a rank-1
    ones-row matmul and splits its drain/store to shorten the tail

Math (token-transposed on host; contraction dim on partitions):
  vp   [k,E]   = (lhsT=vT[D,k], rhs=wv[D,E])            (bv folded into ob!)
  kp^T [E,k]   = (lhsT=wk[D,e], rhs=kT[D,k]) + bk
  qp^T [E,q]   = (lhsT=wq[D,e], rhs=qT[D,q]) * (1/sqrt E) + bq/sqrt(E)
  lgT  [k,q]   = (lhsT=kp^T slice, rhs=qp^T)
  expT [k,q]   = Exp(lgT + mask*NEG)                    (ACT per-partition bias)
  ctx^T[E,q]   = (lhsT=vp slice, rhs=expT) * recip(sum)
  out  [q,D]   = (lhsT=ctx^T slice, rhs=ow[E,D]) + ob_eff
where ob_eff = out_b + wv_b @ out_w (softmax rows sum to 1, so the vp bias
contributes exactly bv @ ow to every output row).
"""

import numpy as np
import ml_dtypes

P = 128
NEG = -1.0e9
BF16 = np.dtype(ml_dtypes.bfloat16)


def build_proj_nc(D=1024, E=1024, SKH=1024):
    """Launch 1: kp^T and vp for this core's half of its batch's keys."""
    import concourse.mybir as mybir
    import concourse.tile as tile
    from concourse import bacc

    f32 = mybir.dt.float32
    bf16 = mybir.dt.bfloat16
    AF = mybir.ActivationFunctionType

    DT = D // P        # 8
    ET = E // P        # 8
    KTH = SKH // P     # 8 key tiles in this half
    NKC = SKH // 512   # 2 chunks

    nc = bacc.Bacc(trn_type="TRN2")

    kTh = nc.dram_tensor("kTh", [D, SKH], bf16, kind="ExternalInput")[:, :]
    vTh = nc.dram_tensor("vTh", [D, SKH], bf16, kind="ExternalInput")[:, :]
    wk = nc.dram_tensor("wk", [D, E], bf16, kind="ExternalInput")[:, :]
    wv = nc.dram_tensor("wv", [D, E], bf16, kind="ExternalInput")[:, :]
    bk_col = nc.dram_tensor("bk_col", [P, ET], f32, kind="ExternalInput")[:, :]
    vp_o = nc.dram_tensor("vp_o", [KTH, P, E], bf16, kind="ExternalOutput")[:, :, :]
    kp_o = nc.dram_tensor("kp_o", [ET, P, SKH], bf16, kind="ExternalOutput")[:, :, :]

    kT_r = kTh.rearrange("(t p) n -> p t n", p=P)
    vT_r = vTh.rearrange("(t p) n -> p t n", p=P)
    wk_r = wk.rearrange("(t p) n -> p t n", p=P)
    wv_r = wv.rearrange("(t p) n -> p t n", p=P)

    def mm(ps, lhsT, rhs, start, stop):
        nc.tensor.matmul(ps, lhsT, rhs, start=start, stop=stop)

    with tile.TileContext(nc) as tc:
        with tc.tile_pool(name="smalls", bufs=1) as smalls, \
             tc.tile_pool(name="res", bufs=1) as res, \
             tc.tile_pool(name="mmps", bufs=1, space="PSUM") as psp:
            def bank(i, name):
                return psp.tile([P, 512], f32, tag=f"b{i}", name=name)

            vp = res.tile([P, KTH, E], bf16, name="vp")
            kp = res.tile([P, ET, SKH], bf16, name="kp")
            wv_t = res.tile([P, DT, E], bf16, name="wv_t")
            wk_t = res.tile([P, DT, E], bf16, name="wk_t")
            bk_t = smalls.tile([P, ET], f32, name="bkc")
            dummy_t = smalls.tile([P, 64], bf16, name="dummy")

            nc.scalar.dma_start(bk_t[:], bk_col)
            nc.vector.memset(dummy_t[:], 1.0)
            wps = psp.tile([P, 512], f32, tag="b7", name="warmps")
            for w in range(75):
                nc.tensor.matmul(wps[0:64, 0:64], dummy_t[:, 0:64],
                                 dummy_t[:, 0:64], start=True, stop=True)

            with tc.tile_pool(name="vp_st", bufs=2) as vst, \
                 tc.tile_pool(name="kp_st", bufs=2) as kst:
                # ---- vp-half: 2 k-quarters of 512, t-outer ----
                for quarter in range(NKC):
                    k0 = quarter * 512
                    vt_q = vst.tile([P, DT, 512], bf16, tag="vT_s",
                                    name=f"vT_{quarter}")
                    if quarter == 0:
                        for (lo, hi) in ((0, 1), (1, 2), (2, 4), (4, 6),
                                         (6, 8)):
                            nc.sync.dma_start(wv_t[:, lo:hi, :],
                                              wv_r[:, lo:hi, :])
                            nc.sync.dma_start(
                                vt_q[:, lo:hi, :],
                                vT_r[:, lo:hi, k0:k0 + 512])
                    else:
                        nc.sync.dma_start(vt_q[:], vT_r[:, :, k0:k0 + 512])
                    pss = {}
                    for mi in range(4):
                        for n in range(2):
                            pss[(mi, n)] = bank(mi * 2 + n,
                                                f"vpps_{quarter}_{mi}_{n}")
                    for t in range(DT):
                        for mi in range(4):
                            for n in range(2):
                                mm(pss[(mi, n)][:],
                                   vt_q[:, t, mi * P:(mi + 1) * P],
                                   wv_t[:, t, n * 512:(n + 1) * 512],
                                   t == 0, t == DT - 1)
                    for mi in range(4):
                        for n in range(2):
                            kb = quarter * 4 + mi
                            dst = vp[:, kb, n * 512:(n + 1) * 512]
                            if (mi + n) % 2 == 0:
                                nc.scalar.activation(dst, pss[(mi, n)][:],
                                                     AF.Identity)
                            else:
                                nc.vector.tensor_copy(dst, pss[(mi, n)][:])
                    for mi in range(4):
                        kb = quarter * 4 + mi
                        nc.gpsimd.dma_start(vp_o[kb], vp[:, kb, :])

                # background: wk then kT chunks on sync (need order)
                for h in range(DT // 2):
                    nc.sync.dma_start(wk_t[:, 2 * h:2 * h + 2, :],
                                      wk_r[:, 2 * h:2 * h + 2, :])

                # ---- kp-half: 2 chunks of 512 ----
                for n in range(NKC):
                    rhs_t = kst.tile([P, DT, 512], bf16, tag="kT_s",
                                     name=f"kT_{n}")
                    nc.sync.dma_start(rhs_t[:], kT_r[:, :, n * 512:(n + 1) * 512])
                    for m in range(ET):
                        ps = bank(m, f"kpps_{n}_{m}")
                        for t in range(DT):
                            mm(ps[:], wk_t[:, t, m * P:(m + 1) * P],
                               rhs_t[:, t, :], t == 0, t == DT - 1)
                        dst = kp[:, m, n * 512:(n + 1) * 512]
                        if m % 2 == 0:
                            nc.scalar.activation(dst, ps[:], AF.Identity,
                                                 bias=bk_t[:, m:m + 1])
                        else:
                            nc.vector.tensor_scalar_add(dst, ps[:],
                                                        bk_t[:, m:m + 1])
                    eng = nc.gpsimd if n == 0 else nc.scalar
                    for m in range(ET):
                        if (n == NKC - 1) and m >= ET - 2:
                            continue  # stores emitted split below for tail
                        eng.dma_start(kp_o[m][:, n * 512:(n + 1) * 512],
                                      kp[:, m, n * 512:(n + 1) * 512])
                # final two stores on fast queues to shorten the tail
                nc.sync.dma_start(kp_o[ET - 2][:, 512:1024],
                                  kp[:, ET - 2, 512:1024])
                nc.scalar.dma_start(kp_o[ET - 1][:, 512:1024],
                                    kp[:, ET - 1, 512:1024])

    nc.compile()
    return nc


def build_attn_nc(D=1024, E=1024, SK=2048, QSH=1024, QB=512):
    """Launch 2: attention for this core's q shard, full kp/vp as inputs."""
    import concourse.mybir as mybir
    import concourse.tile as tile
    from concourse import bacc

    f32 = mybir.dt.float32
    bf16 = mybir.dt.bfloat16
    AF = mybir.ActivationFunctionType
    ALU = mybir.AluOpType

    DT = D // P
    ET = E // P
    KT = SK // P
    NQB = QSH // QB
    DNB = 512
    MQ = QB // P
    ISCALE = 1.0 / float(np.sqrt(E))

    nc = bacc.Bacc(trn_type="TRN2")

    qT = nc.dram_tensor("qT", [D, QSH], bf16, kind="ExternalInput")[:, :]
    kp_i = nc.dram_tensor("kp_i", [ET, P, SK], bf16, kind="ExternalInput")[:, :, :]
    vp_i = nc.dram_tensor("vp_i", [KT, P, E], bf16, kind="ExternalInput")[:, :, :]
    mask_cols = nc.dram_tensor("mask_cols", [P, KT], f32, kind="ExternalInput")[:, :]
    ones_d = nc.dram_tensor("ones_d", [P, P], bf16, kind="ExternalInput")[:, :]
    ob_row_d = nc.dram_tensor("ob_row", [1, D], bf16, kind="ExternalInput")[:, :]
    wq = nc.dram_tensor("wq", [D, E], bf16, kind="ExternalInput")[:, :]
    ow = nc.dram_tensor("ow", [E, D], bf16, kind="ExternalInput")[:, :]
    bq_col = nc.dram_tensor("bq_col", [P, ET], f32, kind="ExternalInput")[:, :]
    ob_bc = nc.dram_tensor("ob_bc", [P, D], f32, kind="ExternalInput")[:, :]
    out = nc.dram_tensor("out", [QSH, D], f32, kind="ExternalOutput")[:, :]

    qT_r = qT.rearrange("(t p) n -> p t n", p=P)
    wq_r = wq.rearrange("(t p) n -> p t n", p=P)
    ow_r = ow.rearrange("(t p) n -> p t n", p=P)

    def mm(ps, lhsT, rhs, start, stop):
        nc.tensor.matmul(ps, lhsT, rhs, start=start, stop=stop)

    with tile.TileContext(nc) as tc:
        with tc.tile_pool(name="smalls", bufs=1) as smalls, \
             tc.tile_pool(name="bigres", bufs=1) as bigres, \
             tc.tile_pool(name="mmps", bufs=1, space="PSUM") as psp:
            def bank(i, name):
                return psp.tile([P, 512], f32, tag=f"b{i}", name=name)

            vp = bigres.tile([P, KT, E], bf16, name="vp")
            kp = bigres.tile([P, ET, SK], bf16, name="kp")
            wq_t = bigres.tile([P, DT, E], bf16, name="wq_t")
            ow_t = bigres.tile([P, ET, D], bf16, name="ow_t")
            qT_sb = bigres.tile([P, DT, QSH], bf16, name="qT_sb")

            mask_t = smalls.tile([P, KT], f32, name="maskc")
            bq_t = smalls.tile([P, ET], f32, name="bqc")
            ones_t = smalls.tile([P, P], bf16, name="ones")
            ob_row = smalls.tile([1, D], bf16, name="ob_row")
            ob_t = smalls.tile([P, D], f32, name="ob_t")
            dummy_t = smalls.tile([P, 64], bf16, name="dummy")
            recip_ts = [smalls.tile([P, QB], f32, name=f"recip{i}")
                        for i in range(NQB)]

            nc.scalar.dma_start(bq_t[:], bq_col)
            nc.scalar.dma_start(mask_t[:], mask_cols)
            nc.scalar.dma_start(ones_t[:], ones_d)
            nc.scalar.dma_start(ob_row[:], ob_row_d)
            nc.scalar.dma_start(ob_t[:], ob_bc)

            # need-order loads on sync: wq + qT first (qp), then kp (logits),
            # vp (ctx), ow (out)
            for (lo, hi) in ((0, 1), (1, 2), (2, 4), (4, 6), (6, 8)):
                nc.sync.dma_start(wq_t[:, lo:hi, :], wq_r[:, lo:hi, :])
                nc.sync.dma_start(qT_sb[:, lo:hi, 0:QB],
                                  qT_r[:, lo:hi, 0:QB])
            for h in range(DT // 2):
                nc.sync.dma_start(qT_sb[:, 2 * h:2 * h + 2, QB:QSH],
                                  qT_r[:, 2 * h:2 * h + 2, QB:QSH])
            for m in range(ET):
                nc.sync.dma_start(kp[:, m, :], kp_i[m])
            for kb2 in range(KT // 2):
                nc.sync.dma_start(vp[:, 2 * kb2, :], vp_i[2 * kb2])
                nc.sync.dma_start(vp[:, 2 * kb2 + 1, :], vp_i[2 * kb2 + 1])
            for h in range(ET // 2):
                nc.sync.dma_start(ow_t[:, 2 * h:2 * h + 2, :],
                                  ow_r[:, 2 * h:2 * h + 2, :])

            nc.vector.memset(dummy_t[:], 1.0)
            wps = psp.tile([P, 512], f32, tag="b7", name="warmps")
            for w in range(76):
                nc.tensor.matmul(wps[0:64, 0:64], dummy_t[:, 0:64],
                                 dummy_t[:, 0:64], start=True, stop=True)

            with tc.tile_pool(name="qp_sb", bufs=1) as qppool, \
                 tc.tile_pool(name="exp_sb", bufs=1) as exppool, \
                 tc.tile_pool(name="sum4_sb", bufs=1) as sum4pool, \
                 tc.tile_pool(name="ctx_sbp", bufs=1) as ctxpool, \
                 tc.tile_pool(name="out_sb", bufs=6) as outpool:
                # -- qp for BOTH q-blocks, t-outer in supply-matched waves
                # so the matmuls pace with the wq/qT DMA stream while the kp
                # load proceeds underneath --
                qps = []
                for qb in range(NQB):
                    q0 = qb * QB
                    qp = qppool.tile([P, ET, QB], bf16, tag=f"qp{qb}",
                                     name=f"qp{qb}")
                    qps.append(qp)
                    for wave, wbanks in (((0, 1, 2, 3, 4, 5, 6),
                                          (4, 5, 6, 0, 1, 2, 3)),
                                         ((7,), (4,))):
                        pss = {}
                        for i, m in enumerate(wave):
                            pss[m] = bank(wbanks[i], f"qpps{qb}_{m}")
                        for t in range(DT):
                            for m in wave:
                                mm(pss[m][:], wq_t[:, t, m * P:(m + 1) * P],
                                   qT_sb[:, t, q0:q0 + QB],
                                   t == 0, t == DT - 1)
                        for m in wave:
                            if m % 2 == 0:
                                nc.scalar.activation(qp[:, m, :], pss[m][:],
                                                     AF.Identity,
                                                     bias=bq_t[:, m:m + 1],
                                                     scale=ISCALE)
                            else:
                                nc.vector.tensor_scalar(
                                    qp[:, m, :], pss[m][:], ISCALE,
                                    bq_t[:, m:m + 1], ALU.mult, ALU.add)

                for qb in range(NQB):
                    q0 = qb * QB
                    qp = qps[qb]

                    expT = exppool.tile([P, KT, QB], bf16, tag="exp",
                                        name=f"exp{qb}")
                    sum4 = sum4pool.tile([P, 4, QB], bf16, tag="sum4",
                                         name=f"sum4_{qb}")
                    tmp2 = sum4pool.tile([P, 2, QB], bf16, tag="tmp2",
                                         name=f"tmp2_{qb}")
                    s_ps = bank(3, f"sps{qb}")
                    for kb in range(KT):
                        ps = bank(kb % 3, f"lgps{qb}_{kb}")
                        for e in range(ET):
                            mm(ps[:], kp[:, e, kb * P:(kb + 1) * P],
                               qp[:, e, :], e == 0, e == ET - 1)
                        nc.scalar.activation(expT[:, kb, :], ps[:], AF.Exp,
                                             bias=mask_t[:, kb:kb + 1])
                        if kb % 4 == 3:
                            g = kb // 4
                            nc.vector.tensor_add(tmp2[:, 0, :],
                                                 expT[:, 4 * g, :],
                                                 expT[:, 4 * g + 1, :])
                            nc.vector.tensor_add(tmp2[:, 1, :],
                                                 expT[:, 4 * g + 2, :],
                                                 expT[:, 4 * g + 3, :])
                            nc.vector.tensor_add(sum4[:, g, :],
                                                 tmp2[:, 0, :],
                                                 tmp2[:, 1, :])
                        if kb == 9:
                            nc.vector.tensor_add(sum4[:, 0, :],
                                                 sum4[:, 0, :],
                                                 sum4[:, 1, :])

                    ctx_sb = ctxpool.tile([P, ET, QB], bf16, tag="ctx",
                                          name=f"ctx{qb}")
                    for e in range(ET):
                        ps = bank(4 + e % 3, f"ctxps{qb}_{e}")
                        for kb in range(KT):
                            mm(ps[:], vp[:, kb, e * P:(e + 1) * P],
                               expT[:, kb, :], kb == 0, kb == KT - 1)
                            if e == 0 and kb == 1:
                                nc.vector.tensor_add(sum4[:, 2, :],
                                                     sum4[:, 2, :],
                                                     sum4[:, 3, :])
                            if e == 0 and kb == 3:
                                nc.vector.tensor_add(sum4[:, 0, :],
                                                     sum4[:, 0, :],
                                                     sum4[:, 2, :])
                            if e == 0 and kb == 13:
                                mm(s_ps[:], ones_t[:], sum4[:, 0, :],
                                   True, True)
                        if e == 0:
                            nc.vector.reciprocal(recip_ts[qb][:], s_ps[:])
                        nc.vector.tensor_mul(ctx_sb[:, e, :], ps[:],
                                             recip_ts[qb][:])

                    for nd in range(D // DNB):
                        for mq in range(MQ):
                            idx = nd * MQ + mq
                            last = (qb == NQB - 1 and idx == 2 * MQ - 1)
                            rows = out[q0 + mq * P: q0 + (mq + 1) * P,
                                       nd * DNB:(nd + 1) * DNB]
                            ot = outpool.tile([P, DNB], f32, tag="ot",
                                              name=f"ot{qb}_{idx}")
                            if not last:
                                ps = bank(idx % 3, f"ops{qb}_{nd}_{mq}")
                                for e in range(ET):
                                    mm(ps[:],
                                       ctx_sb[:, e, mq * P:(mq + 1) * P],
                                       ow_t[:, e, nd * DNB:(nd + 1) * DNB],
                                       e == 0, e == ET - 1)
                                nc.vector.tensor_add(
                                    ot[:], ps[:],
                                    ob_t[:, nd * DNB:(nd + 1) * DNB])
                                nc.gpsimd.dma_start(rows, ot[:])
                            else:
                                H = DNB // 2
                                for hh in range(2):
                                    o0 = nd * DNB + hh * H
                                    ph = bank((idx + hh) % 3,
                                              f"ops{qb}_{nd}_{mq}_{hh}")
                                    for e in range(ET):
                                        mm(ph[:, :H],
                                           ctx_sb[:, e, mq * P:(mq + 1) * P],
                                           ow_t[:, e, o0:o0 + H],
                                           e == 0, False)
                                    nc.tensor.matmul(
                                        ph[:, :H], ones_t[0:1, 0:P],
                                        ob_row[0:1, o0:o0 + H],
                                        start=False, stop=True)
                                    sl = slice(hh * H, hh * H + H)
                                    if hh == 0:
                                        nc.vector.tensor_copy(ot[:, sl],
                                                              ph[:, :H])
                                        nc.sync.dma_start(rows[:, sl],
                                                          ot[:, sl])
                                    else:
                                        nc.scalar.activation(ot[:, sl],
                                                             ph[:, :H],
                                                             AF.Identity)
                                        nc.scalar.dma_start(rows[:, sl],
                                                            ot[:, sl])

    nc.compile()
    return nc


def kernel(v, k, q, mask, wq_w, wq_b, wk_w, wk_b, wv_w, wv_b, out_w, out_b):
    import os
    from concourse.bass_utils import run_bass_kernel_spmd

    B, S, D = 4, 2048, 1024
    E, QSH, SKH = 1024, 1024, 1024
    ET, KT = E // P, S // P
    f = np.float32
    ISCALE = 1.0 / float(np.sqrt(E))

    if "proj" not in _NC_CACHE:
        _NC_CACHE["proj"] = build_proj_nc(D=D, E=E, SKH=SKH)
        _NC_CACHE["attn"] = build_attn_nc(D=D, E=E, SK=S, QSH=QSH, QB=512)
    nc1 = _NC_CACHE["proj"]
    nc2 = _NC_CACHE["attn"]
    _NC_CACHE["ncs"] = [nc1, nc2]
    _NC_CACHE["nc"] = nc2

    trace = bool(int(os.environ.get("BASS_KERNEL_TRACE", "0")))

    # ---- launch 1: K/V projections, each core does half a batch's keys ----
    wk_bf = np.ascontiguousarray(np.asarray(wk_w, f).astype(BF16))
    wv_bf = np.ascontiguousarray(np.asarray(wv_w, f).astype(BF16))
    bk_col = np.ascontiguousarray(np.asarray(wk_b, f).reshape(ET, P).T)
    in_maps1 = []
    for c in range(8):
        b, h = divmod(c, 2)
        kTh = np.ascontiguousarray(
            np.asarray(k[b, h * SKH:(h + 1) * SKH, :], f).T.astype(BF16))
        vTh = np.ascontiguousarray(
            np.asarray(v[b, h * SKH:(h + 1) * SKH, :], f).T.astype(BF16))
        in_maps1.append(dict(kTh=kTh, vTh=vTh, wk=wk_bf, wv=wv_bf,
                             bk_col=bk_col))
    res1 = run_bass_kernel_spmd(nc1, in_maps1, core_ids=list(range(8)),
                                trace=trace)

    # ---- host gather: assemble each batch's full kp/vp from its halves ----
    kp_full, vp_full = [], []
    for b in range(B):
        lo, hi = res1.results[2 * b], res1.results[2 * b + 1]
        kp_full.append(np.ascontiguousarray(
            np.concatenate([lo["kp_o"], hi["kp_o"]], axis=2)))
        vp_full.append(np.ascontiguousarray(
            np.concatenate([lo["vp_o"], hi["vp_o"]], axis=0)))

    # ---- launch 2: attention over each core's q shard ----
    wq_bf = np.ascontiguousarray(np.asarray(wq_w, f).astype(BF16))
    ow_bf = np.ascontiguousarray(np.asarray(out_w, f).astype(BF16))
    bq_col = np.ascontiguousarray(
        (np.asarray(wq_b, f) * ISCALE).reshape(ET, P).T)
    ob_eff = np.asarray(out_b, f) + np.asarray(wv_b, f) @ np.asarray(out_w, f)
    ob_bc = np.ascontiguousarray(np.broadcast_to(ob_eff, (P, len(out_b))))
    ob_row = np.ascontiguousarray(ob_eff.reshape(1, -1).astype(BF16))
    ones_arr = np.ones((P, P), BF16)
    in_maps2 = []
    for c in range(8):
        b, h = divmod(c, 2)
        qTc = np.ascontiguousarray(
            np.asarray(q[b, h * QSH:(h + 1) * QSH, :], f).T.astype(BF16))
        mc = np.ascontiguousarray(
            (np.asarray(mask[b, 0], f) * NEG).reshape(KT, P).T)
        in_maps2.append(dict(qT=qTc, kp_i=kp_full[b], vp_i=vp_full[b],
                             mask_cols=mc, ones_d=ones_arr, ob_row=ob_row,
                             wq=wq_bf, ow=ow_bf, bq_col=bq_col, ob_bc=ob_bc))
    res2 = run_bass_kernel_spmd(nc2, in_maps2, core_ids=list(range(8)),
                                trace=trace)

    if trace:
        t1 = res1.exec_time_ns or 0
        t2 = res2.exec_time_ns or 0
        print(f"HW exec time: {t1 + t2} ns")
        _NC_CACHE["last_exec_time_ns"] = t1 + t2

    outp = np.empty((B, S, D), np.float32)
    for c in range(8):
        b, h = divmod(c, 2)
        outp[b, h * QSH:(h + 1) * QSH, :] = res2.results[c]["out"]
    return outp


_NC_CACHE = {}


# revision 41
# speedup vs baseline: 1.0140x; 1.0140x over previous
"""Single-head attention (B=4, S=2048, D=E=1024) on 8 trn2 NeuronCores.

Two-launch tensor-parallel scheme (the sharding hint's Megatron-style split,
with the collective realized as the host gather/unshard step the kernel()
contract provides between launches):

  Launch 1 (proj): data-parallel over (batch, k-half). Each core computes
    the K/V projections for HALF of its batch's keys -- kp^T[:, k-half] and
    vp[k-half, :] -- so nothing is computed twice. 256 matmuls/core.
  Host: concatenates the two halves of each batch's kp/vp (the all-gather).
  Launch 2 (attn): data-parallel over (batch, q-half). Each core loads the
    full kp/vp for its batch plus its 1024-row q shard and runs
    qp -> logits -> softmax -> ctx -> out. 770 matmuls/core.

Both launches use the techniques that made the single-launch version run as
a gapless PE stream at 213 ns per 512-row bf16 matmul:
  - all-bf16 operands (same steady-state PE rate as f32r, half the bytes)
  - ONE psum pool with 8 fixed bank tags per module (per-phase pools would
    insert full-pool barrier waits on the PE sequencer; shared tags make
    the hazards per-bank WARs that resolve early)
  - drains alternate Act/DVE so the last drain gating a successor phase
    has ~0.7us latency, covered by the successor's first matmuls
  - tiny warm-up matmuls through the DMA-latency startup window so the
    cost model's PE clock ramp (0.65->1.2->2.4 GHz over 3us) completes
    before the real stream begins
  - one sync-queue DMA stream in need-order (FIFO self-throttles
    background loads behind the critical ones)
  - softmax sum via DVE-presummed exp tiles (1 matmul per q-block)
  - out projection fused per q-block; final tile folds ob via a rank-1
    ones-row matmul and splits its drain/store to shorten the tail

Math (token-transposed on host; contraction dim on partitions):
  vp   [k,E]   = (lhsT=vT[D,k], rhs=wv[D,E])            (bv folded into ob!)
  kp^T [E,k]   = (lhsT=wk[D,e], rhs=kT[D,k]) + bk
  qp^T [E,q]   = (lhsT=wq[D,e], rhs=qT[D,q]) * (1/sqrt E) + bq/sqrt(E)
  lgT  [k,q]   = (lhsT=kp^T slice, rhs=qp^T)
  expT [k,q]   = Exp(lgT + mask*NEG)                    (ACT per-partition bias)
  ctx^T[E,q]   = (lhsT=vp slice, rhs=expT) * recip(sum)
  out  [q,D]   = (lhsT=ctx^T slice, rhs=ow[E,D]) + ob_eff
where ob_eff = out_b + wv_b @ out_w (softmax rows sum to 1, so the vp bias
contributes exactly bv @ ow to every output row).
"""

import numpy as np
import ml_dtypes

P = 128
NEG = -1.0e9
BF16 = np.dtype(ml_dtypes.bfloat16)


def build_proj_nc(D=1024, E=1024, SKH=1024):
    """Launch 1: kp^T and vp for this core's half of its batch's keys."""
    import concourse.mybir as mybir
    import concourse.tile as tile
    from concourse import bacc

    f32 = mybir.dt.float32
    bf16 = mybir.dt.bfloat16
    AF = mybir.ActivationFunctionType

    DT = D // P        # 8
    ET = E // P        # 8
    KTH = SKH // P     # 8 key tiles in this half
    NKC = SKH // 512   # 2 chunks

    nc = bacc.Bacc(trn_type="TRN2")

    kTh = nc.dram_tensor("kTh", [D, SKH], bf16, kind="ExternalInput")[:, :]
    vTh = nc.dram_tensor("vTh", [D, SKH], bf16, kind="ExternalInput")[:, :]
    wk = nc.dram_tensor("wk", [D, E], bf16, kind="ExternalInput")[:, :]
    wv = nc.dram_tensor("wv", [D, E], bf16, kind="ExternalInput")[:, :]
    bk_col = nc.dram_tensor("bk_col", [P, ET], f32, kind="ExternalInput")[:, :]
    vp_o = nc.dram_tensor("vp_o", [KTH, P, E], bf16, kind="ExternalOutput")[:, :, :]
    kp_o = nc.dram_tensor("kp_o", [ET, P, SKH], bf16, kind="ExternalOutput")[:, :, :]

    kT_r = kTh.rearrange("(t p) n -> p t n", p=P)
    vT_r = vTh.rearrange("(t p) n -> p t n", p=P)
    wk_r = wk.rearrange("(t p) n -> p t n", p=P)
    wv_r = wv.rearrange("(t p) n -> p t n", p=P)

    def mm(ps, lhsT, rhs, start, stop):
        nc.tensor.matmul(ps, lhsT, rhs, start=start, stop=stop)

    with tile.TileContext(nc) as tc:
        with tc.tile_pool(name="smalls", bufs=1) as smalls, \
             tc.tile_pool(name="res", bufs=1) as res, \
             tc.tile_pool(name="mmps", bufs=1, space="PSUM") as psp:
            def bank(i, name):
                return psp.tile([P, 512], f32, tag=f"b{i}", name=name)

            vp = res.tile([P, KTH, E], bf16, name="vp")
            kp = res.tile([P, ET, SKH], bf16, name="kp")
            wv_t = res.tile([P, DT, E], bf16, name="wv_t")
            wk_t = res.tile([P, DT, E], bf16, name="wk_t")
            bk_t = smalls.tile([P, ET], f32, name="bkc")
            dummy_t = smalls.tile([P, 64], bf16, name="dummy")

            nc.scalar.dma_start(bk_t[:], bk_col)
            nc.vector.memset(dummy_t[:], 1.0)
            wps = psp.tile([P, 512], f32, tag="b7", name="warmps")
            for w in range(88):
                nc.tensor.matmul(wps[0:64, 0:64], dummy_t[:, 0:64],
                                 dummy_t[:, 0:64], start=True, stop=True)

            with tc.tile_pool(name="vp_st", bufs=2) as vst, \
                 tc.tile_pool(name="kp_st", bufs=2) as kst:
                # ---- vp-half: 2 k-quarters of 512, t-outer ----
                for quarter in range(NKC):
                    k0 = quarter * 512
                    vt_q = vst.tile([P, DT, 512], bf16, tag="vT_s",
                                    name=f"vT_{quarter}")
                    if quarter == 0:
                        for h in range(4):
                            nc.sync.dma_start(wv_t[:, 2 * h:2 * h + 2, :],
                                              wv_r[:, 2 * h:2 * h + 2, :])
                            nc.sync.dma_start(
                                vt_q[:, 2 * h:2 * h + 2, :],
                                vT_r[:, 2 * h:2 * h + 2, k0:k0 + 512])
                    else:
                        nc.sync.dma_start(vt_q[:], vT_r[:, :, k0:k0 + 512])
                    pss = {}
                    for mi in range(4):
                        for n in range(2):
                            pss[(mi, n)] = bank(mi * 2 + n,
                                                f"vpps_{quarter}_{mi}_{n}")
                    for t in range(DT):
                        for mi in range(4):
                            for n in range(2):
                                mm(pss[(mi, n)][:],
                                   vt_q[:, t, mi * P:(mi + 1) * P],
                                   wv_t[:, t, n * 512:(n + 1) * 512],
                                   t == 0, t == DT - 1)
                    for mi in range(4):
                        for n in range(2):
                            kb = quarter * 4 + mi
                            dst = vp[:, kb, n * 512:(n + 1) * 512]
                            if (mi + n) % 2 == 0:
                                nc.scalar.activation(dst, pss[(mi, n)][:],
                                                     AF.Identity)
                            else:
                                nc.vector.tensor_copy(dst, pss[(mi, n)][:])
                    for mi in range(4):
                        kb = quarter * 4 + mi
                        nc.gpsimd.dma_start(vp_o[kb], vp[:, kb, :])

                # background: wk then kT chunks on sync (need order)
                for h in range(DT // 2):
                    nc.sync.dma_start(wk_t[:, 2 * h:2 * h + 2, :],
                                      wk_r[:, 2 * h:2 * h + 2, :])

                # ---- kp-half: 2 chunks of 512 ----
                for n in range(NKC):
                    rhs_t = kst.tile([P, DT, 512], bf16, tag="kT_s",
                                     name=f"kT_{n}")
                    nc.sync.dma_start(rhs_t[:], kT_r[:, :, n * 512:(n + 1) * 512])
                    for m in range(ET):
                        ps = bank(m, f"kpps_{n}_{m}")
                        for t in range(DT):
                            mm(ps[:], wk_t[:, t, m * P:(m + 1) * P],
                               rhs_t[:, t, :], t == 0, t == DT - 1)
                        dst = kp[:, m, n * 512:(n + 1) * 512]
                        if m % 2 == 0:
                            nc.scalar.activation(dst, ps[:], AF.Identity,
                                                 bias=bk_t[:, m:m + 1])
                        else:
                            nc.vector.tensor_scalar_add(dst, ps[:],
                                                        bk_t[:, m:m + 1])
                    eng = nc.gpsimd if n == 0 else nc.scalar
                    for m in range(ET):
                        if (n == NKC - 1) and m >= ET - 2:
                            continue  # stores emitted split below for tail
                        eng.dma_start(kp_o[m][:, n * 512:(n + 1) * 512],
                                      kp[:, m, n * 512:(n + 1) * 512])
                # final two stores on fast queues to shorten the tail
                nc.sync.dma_start(kp_o[ET - 2][:, 512:1024],
                                  kp[:, ET - 2, 512:1024])
                nc.scalar.dma_start(kp_o[ET - 1][:, 512:1024],
                                    kp[:, ET - 1, 512:1024])

    nc.compile()
    return nc


def build_attn_nc(D=1024, E=1024, SK=2048, QSH=1024, QB=512):
    """Launch 2: attention for this core's q shard, full kp/vp as inputs."""
    import concourse.mybir as mybir
    import concourse.tile as tile
    from concourse import bacc

    f32 = mybir.dt.float32
    bf16 = mybir.dt.bfloat16
    AF = mybir.ActivationFunctionType
    ALU = mybir.AluOpType

    DT = D // P
    ET = E // P
    KT = SK // P
    NQB = QSH // QB
    DNB = 512
    MQ = QB // P
    ISCALE = 1.0 / float(np.sqrt(E))

    nc = bacc.Bacc(trn_type="TRN2")

    qT = nc.dram_tensor("qT", [D, QSH], bf16, kind="ExternalInput")[:, :]
    kp_i = nc.dram_tensor("kp_i", [ET, P, SK], bf16, kind="ExternalInput")[:, :, :]
    vp_i = nc.dram_tensor("vp_i", [KT, P, E], bf16, kind="ExternalInput")[:, :, :]
    mask_cols = nc.dram_tensor("mask_cols", [P, KT], f32, kind="ExternalInput")[:, :]
    ones_d = nc.dram_tensor("ones_d", [P, P], bf16, kind="ExternalInput")[:, :]
    ob_row_d = nc.dram_tensor("ob_row", [1, D], bf16, kind="ExternalInput")[:, :]
    wq = nc.dram_tensor("wq", [D, E], bf16, kind="ExternalInput")[:, :]
    ow = nc.dram_tensor("ow", [E, D], bf16, kind="ExternalInput")[:, :]
    bq_col = nc.dram_tensor("bq_col", [P, ET], f32, kind="ExternalInput")[:, :]
    ob_bc = nc.dram_tensor("ob_bc", [P, D], f32, kind="ExternalInput")[:, :]
    out = nc.dram_tensor("out", [QSH, D], f32, kind="ExternalOutput")[:, :]

    qT_r = qT.rearrange("(t p) n -> p t n", p=P)
    wq_r = wq.rearrange("(t p) n -> p t n", p=P)
    ow_r = ow.rearrange("(t p) n -> p t n", p=P)

    def mm(ps, lhsT, rhs, start, stop):
        nc.tensor.matmul(ps, lhsT, rhs, start=start, stop=stop)

    with tile.TileContext(nc) as tc:
        with tc.tile_pool(name="smalls", bufs=1) as smalls, \
             tc.tile_pool(name="bigres", bufs=1) as bigres, \
             tc.tile_pool(name="mmps", bufs=1, space="PSUM") as psp:
            def bank(i, name):
                return psp.tile([P, 512], f32, tag=f"b{i}", name=name)

            vp = bigres.tile([P, KT, E], bf16, name="vp")
            kp = bigres.tile([P, ET, SK], bf16, name="kp")
            wq_t = bigres.tile([P, DT, E], bf16, name="wq_t")
            ow_t = bigres.tile([P, ET, D], bf16, name="ow_t")
            qT_sb = bigres.tile([P, DT, QSH], bf16, name="qT_sb")

            mask_t = smalls.tile([P, KT], f32, name="maskc")
            bq_t = smalls.tile([P, ET], f32, name="bqc")
            ones_t = smalls.tile([P, P], bf16, name="ones")
            ob_row = smalls.tile([1, D], bf16, name="ob_row")
            ob_t = smalls.tile([P, D], f32, name="ob_t")
            dummy_t = smalls.tile([P, 64], bf16, name="dummy")
            recip_ts = [smalls.tile([P, QB], f32, name=f"recip{i}")
                        for i in range(NQB)]

            nc.scalar.dma_start(bq_t[:], bq_col)
            nc.scalar.dma_start(mask_t[:], mask_cols)
            nc.scalar.dma_start(ones_t[:], ones_d)
            nc.scalar.dma_start(ob_row[:], ob_row_d)
            nc.scalar.dma_start(ob_t[:], ob_bc)

            # need-order loads on sync: wq + qT first (qp), then kp (logits),
            # vp (ctx), ow (out)
            for h in range(DT // 2):
                nc.sync.dma_start(wq_t[:, 2 * h:2 * h + 2, :],
                                  wq_r[:, 2 * h:2 * h + 2, :])
                nc.sync.dma_start(qT_sb[:, 2 * h:2 * h + 2, 0:QB],
                                  qT_r[:, 2 * h:2 * h + 2, 0:QB])
            for h in range(DT // 2):
                nc.sync.dma_start(qT_sb[:, 2 * h:2 * h + 2, QB:QSH],
                                  qT_r[:, 2 * h:2 * h + 2, QB:QSH])
            for m in range(ET):
                nc.sync.dma_start(kp[:, m, :], kp_i[m])
            for kb2 in range(KT // 2):
                nc.sync.dma_start(vp[:, 2 * kb2, :], vp_i[2 * kb2])
                nc.sync.dma_start(vp[:, 2 * kb2 + 1, :], vp_i[2 * kb2 + 1])
            for h in range(ET // 2):
                nc.sync.dma_start(ow_t[:, 2 * h:2 * h + 2, :],
                                  ow_r[:, 2 * h:2 * h + 2, :])

            nc.vector.memset(dummy_t[:], 1.0)
            wps = psp.tile([P, 512], f32, tag="b7", name="warmps")
            for w in range(76):
                nc.tensor.matmul(wps[0:64, 0:64], dummy_t[:, 0:64],
                                 dummy_t[:, 0:64], start=True, stop=True)

            with tc.tile_pool(name="qp_sb", bufs=1) as qppool, \
                 tc.tile_pool(name="exp_sb", bufs=1) as exppool, \
                 tc.tile_pool(name="sum4_sb", bufs=1) as sum4pool, \
                 tc.tile_pool(name="ctx_sbp", bufs=1) as ctxpool, \
                 tc.tile_pool(name="out_sb", bufs=6) as outpool:
                # -- qp for BOTH q-blocks, t-outer in supply-matched waves
                # so the matmuls pace with the wq/qT DMA stream while the kp
                # load proceeds underneath --
                qps = []
                for qb in range(NQB):
                    q0 = qb * QB
                    qp = qppool.tile([P, ET, QB], bf16, tag=f"qp{qb}",
                                     name=f"qp{qb}")
                    qps.append(qp)
                    for wave, wbanks in (((0, 1, 2, 3, 4, 5, 6),
                                          (4, 5, 6, 0, 1, 2, 3)),
                                         ((7,), (4,))):
                        pss = {}
                        for i, m in enumerate(wave):
                            pss[m] = bank(wbanks[i], f"qpps{qb}_{m}")
                        for t in range(DT):
                            for m in wave:
                                mm(pss[m][:], wq_t[:, t, m * P:(m + 1) * P],
                                   qT_sb[:, t, q0:q0 + QB],
                                   t == 0, t == DT - 1)
                        for m in wave:
                            if m % 2 == 0:
                                nc.scalar.activation(qp[:, m, :], pss[m][:],
                                                     AF.Identity,
                                                     bias=bq_t[:, m:m + 1],
                                                     scale=ISCALE)
                            else:
                                nc.vector.tensor_scalar(
                                    qp[:, m, :], pss[m][:], ISCALE,
                                    bq_t[:, m:m + 1], ALU.mult, ALU.add)

                for qb in range(NQB):
                    q0 = qb * QB
                    qp = qps[qb]

                    expT = exppool.tile([P, KT, QB], bf16, tag="exp",
                                        name=f"exp{qb}")
                    sum4 = sum4pool.tile([P, 4, QB], bf16, tag="sum4",
                                         name=f"sum4_{qb}")
                    tmp2 = sum4pool.tile([P, 2, QB], bf16, tag="tmp2",
                                         name=f"tmp2_{qb}")
                    s_ps = bank(3, f"sps{qb}")
                    for kb in range(KT):
                        ps = bank(kb % 3, f"lgps{qb}_{kb}")
                        for e in range(ET):
                            mm(ps[:], kp[:, e, kb * P:(kb + 1) * P],
                               qp[:, e, :], e == 0, e == ET - 1)
                        nc.scalar.activation(expT[:, kb, :], ps[:], AF.Exp,
                                             bias=mask_t[:, kb:kb + 1])
                        if kb % 4 == 3:
                            g = kb // 4
                            nc.vector.tensor_add(tmp2[:, 0, :],
                                                 expT[:, 4 * g, :],
                                                 expT[:, 4 * g + 1, :])
                            nc.vector.tensor_add(tmp2[:, 1, :],
                                                 expT[:, 4 * g + 2, :],
                                                 expT[:, 4 * g + 3, :])
                            nc.vector.tensor_add(sum4[:, g, :],
                                                 tmp2[:, 0, :],
                                                 tmp2[:, 1, :])
                        if kb == 9:
                            nc.vector.tensor_add(sum4[:, 0, :],
                                                 sum4[:, 0, :],
                                                 sum4[:, 1, :])

                    ctx_sb = ctxpool.tile([P, ET, QB], bf16, tag="ctx",
                                          name=f"ctx{qb}")
                    for e in range(ET):
                        ps = bank(4 + e % 3, f"ctxps{qb}_{e}")
                        for kb in range(KT):
                            mm(ps[:], vp[:, kb, e * P:(e + 1) * P],
                               expT[:, kb, :], kb == 0, kb == KT - 1)
                            if e == 0 and kb == 1:
                                nc.vector.tensor_add(sum4[:, 2, :],
                                                     sum4[:, 2, :],
                                                     sum4[:, 3, :])
                            if e == 0 and kb == 3:
                                nc.vector.tensor_add(sum4[:, 0, :],
                                                     sum4[:, 0, :],
                                                     sum4[:, 2, :])
                            if e == 0 and kb == 13:
                                mm(s_ps[:], ones_t[:], sum4[:, 0, :],
                                   True, True)
                        if e == 0:
                            nc.vector.reciprocal(recip_ts[qb][:], s_ps[:])
                        nc.vector.tensor_mul(ctx_sb[:, e, :], ps[:],
                                             recip_ts[qb][:])

                    for nd in range(D // DNB):
                        for mq in range(MQ):
                            idx = nd * MQ + mq
                            last = (qb == NQB - 1 and idx == 2 * MQ - 1)
                            rows = out[q0 + mq * P: q0 + (mq + 1) * P,
                                       nd * DNB:(nd + 1) * DNB]
                            ot = outpool.tile([P, DNB], f32, tag="ot",
                                              name=f"ot{qb}_{idx}")
                            if not last:
                                ps = bank(idx % 3, f"ops{qb}_{nd}_{mq}")
                                for e in range(ET):
                                    mm(ps[:],
                                       ctx_sb[:, e, mq * P:(mq + 1) * P],
                                       ow_t[:, e, nd * DNB:(nd + 1) * DNB],
                                       e == 0, e == ET - 1)
                                nc.vector.tensor_add(
                                    ot[:], ps[:],
                                    ob_t[:, nd * DNB:(nd + 1) * DNB])
                                nc.gpsimd.dma_start(rows, ot[:])
                            else:
                                H = DNB // 2
                                for hh in range(2):
                                    o0 = nd * DNB + hh * H
                                    ph = bank((idx + hh) % 3,
                                              f"ops{qb}_{nd}_{mq}_{hh}")
                                    for e in range(ET):
                                        mm(ph[:, :H],
                                           ctx_sb[:, e, mq * P:(mq + 1) * P],
                                           ow_t[:, e, o0:o0 + H],
                                           e == 0, False)
                                    nc.tensor.matmul(
                                        ph[:, :H], ones_t[0:1, 0:P],
                                        ob_row[0:1, o0:o0 + H],
                                        start=False, stop=True)
                                    sl = slice(hh * H, hh * H + H)
                                    if hh == 0:
                                        nc.vector.tensor_copy(ot[:, sl],
                                                              ph[:, :H])
                                        nc.sync.dma_start(rows[:, sl],
                                                          ot[:, sl])
                                    else:
                                        nc.scalar.activation(ot[:, sl],
                                                             ph[:, :H],
                                                             AF.Identity)
                                        nc.scalar.dma_start(rows[:, sl],
                                                            ot[:, sl])

    nc.compile()
    return nc


def kernel(v, k, q, mask, wq_w, wq_b, wk_w, wk_b, wv_w, wv_b, out_w, out_b):
    import os
    from concourse.bass_utils import run_bass_kernel_spmd

    B, S, D = 4, 2048, 1024
    E, QSH, SKH = 1024, 1024, 1024
    ET, KT = E // P, S // P
    f = np.float32
    ISCALE = 1.0 / float(np.sqrt(E))

    if "proj" not in _NC_CACHE:
        _NC_CACHE["proj"] = build_proj_nc(D=D, E=E, SKH=SKH)
        _NC_CACHE["attn"] = build_attn_nc(D=D, E=E, SK=S, QSH=QSH, QB=512)
    nc1 = _NC_CACHE["proj"]
    nc2 = _NC_CACHE["attn"]
    _NC_CACHE["ncs"] = [nc1, nc2]
    _NC_CACHE["nc"] = nc2

    trace = bool(int(os.environ.get("BASS_KERNEL_TRACE", "0")))

    # ---- launch 1: K/V projections, each core does half a batch's keys ----
    wk_bf = np.ascontiguousarray(np.asarray(wk_w, f).astype(BF16))
    wv_bf = np.ascontiguousarray(np.asarray(wv_w, f).astype(BF16))
    bk_col = np.ascontiguousarray(np.asarray(wk_b, f).reshape(ET, P).T)
    in_maps1 = []
    for c in range(8):
        b, h = divmod(c, 2)
        kTh = np.ascontiguousarray(
            np.asarray(k[b, h * SKH:(h + 1) * SKH, :], f).T.astype(BF16))
        vTh = np.ascontiguousarray(
            np.asarray(v[b, h * SKH:(h + 1) * SKH, :], f).T.astype(BF16))
        in_maps1.append(dict(kTh=kTh, vTh=vTh, wk=wk_bf, wv=wv_bf,
                             bk_col=bk_col))
    res1 = run_bass_kernel_spmd(nc1, in_maps1, core_ids=list(range(8)),
                                trace=trace)

    # ---- host gather: assemble each batch's full kp/vp from its halves ----
    kp_full, vp_full = [], []
    for b in range(B):
        lo, hi = res1.results[2 * b], res1.results[2 * b + 1]
        kp_full.append(np.ascontiguousarray(
            np.concatenate([lo["kp_o"], hi["kp_o"]], axis=2)))
        vp_full.append(np.ascontiguousarray(
            np.concatenate([lo["vp_o"], hi["vp_o"]], axis=0)))

    # ---- launch 2: attention over each core's q shard ----
    wq_bf = np.ascontiguousarray(np.asarray(wq_w, f).astype(BF16))
    ow_bf = np.ascontiguousarray(np.asarray(out_w, f).astype(BF16))
    bq_col = np.ascontiguousarray(
        (np.asarray(wq_b, f) * ISCALE).reshape(ET, P).T)
    ob_eff = np.asarray(out_b, f) + np.asarray(wv_b, f) @ np.asarray(out_w, f)
    ob_bc = np.ascontiguousarray(np.broadcast_to(ob_eff, (P, len(out_b))))
    ob_row = np.ascontiguousarray(ob_eff.reshape(1, -1).astype(BF16))
    ones_arr = np.ones((P, P), BF16)
    in_maps2 = []
    for c in range(8):
        b, h = divmod(c, 2)
        qTc = np.ascontiguousarray(
            np.asarray(q[b, h * QSH:(h + 1) * QSH, :], f).T.astype(BF16))
        mc = np.ascontiguousarray(
            (np.asarray(mask[b, 0], f) * NEG).reshape(KT, P).T)
        in_maps2.append(dict(qT=qTc, kp_i=kp_full[b], vp_i=vp_full[b],
                             mask_cols=mc, ones_d=ones_arr, ob_row=ob_row,
                             wq=wq_bf, ow=ow_bf, bq_col=bq_col, ob_bc=ob_bc))
    res2 = run_bass_kernel_spmd(nc2, in_maps2, core_ids=list(range(8)),
                                trace=trace)

    if trace:
        t1 = res1.exec_time_ns or 0
        t2 = res2.exec_time_ns or 0
        print(f"HW exec time: {t1 + t2} ns")
        _NC_CACHE["last_exec_time_ns"] = t1 + t2

    outp = np.empty((B, S, D), np.float32)
    for c in range(8):
        b, h = divmod(c, 2)
        outp[b, h * QSH:(h + 1) * QSH, :] = res2.results[c]["out"]
    return outp


_NC_CACHE = {}
